# revision 49
# baseline (speedup 1.0000x reference)
"""Trainium2 Bass kernel for nn_CascadedAttention_76836964925817.

Math: the reference module's attention machinery is dead code — softmax over a
size-1 axis is identically 1, so `context = x[0].sum(axis=0)` is a constant
and the layer reduces to the 28-dim nonlinear recurrence

    y[t] = sigmoid(Wo @ y[t-1] + Uo @ x[t-1] + c),   c = Co @ sum_t x[t],
    y[-1] = 0, x[-1] := 0,

solved by Jacobi fixed-point sweeps (the map is a strong contraction:
|sigmoid'| <= 1/4, ||Wo|| ~ 0.53, plus heavy sigmoid saturation from the
large |c|; 3 sweeps reach the fp16 data floor of ~1e-3 rel).

Strategy: **no collectives**.  Every core redundantly computes the whole
problem from an fp16 copy of x (4.2 MB); the host reads core 0's output.
x is rounded on the host with 1-D error diffusion along t (per feature), so
the device-side t-sums reproduce the fp32 sums to ~1 ulp — this keeps the
constant c accurate without shipping a second hi/lo copy of x.

  * x is d-major, one (128, 1+2048) fp16 tile per 128-wide d-chunk (one
    leading zero column so B's shift-by-one is a column offset); chunk DMAs
    split across the two HWDGE queues (SP even chunks, Activation odd) with
    per-chunk tiles so consumers start as soon as their chunk lands.
  * U-phase: per chunk, 4 matmuls (lhsT = padded Uo chunk (128, 32)) write
    the four 512-col t-groups into ONE stacked psum bank at 32-partition
    strides (tile_position=(0, 32g)) — the bank then IS the shifted B.
  * c-path: per-chunk t-sums — 6 chunks on Vector (fused pair-add + reduce,
    fp32 elementwise output), 2 on Scalar (Identity activation with fp32
    accum_out); then 8 tiny accumulating matmuls against group-replicated
    Co weights (128-wide lhsT) put the full 128-partition bias in one psum
    column; GpSimd copies it to SBUF.
  * Sweeps: sweep 0 is one ACT (sigmoid(B + c) straight from the U psum).
    The two z psum banks are pre-loaded with B (eye matmuls, off-path);
    sweeps 1-2 accumulate blockdiag(Wo^T) @ YA (plus a 1-col group-boundary
    matmul) on top and ACT with bias=c.

The kernel is self-contained: shapes/sharding are hardcoded.
"""

import numpy as np

import concourse.bass as bass
import concourse.mybir as mybir
import concourse.tile as tile
from concourse import bacc
from concourse import bass_utils

F32 = mybir.dt.float32
F32R = mybir.dt.float32r
F16 = mybir.dt.float16
U16 = mybir.dt.uint16
AF = mybir.ActivationFunctionType
ALU = mybir.AluOpType

T, D, V = 2048, 1024, 28
N_CORES = 8
G = 4                      # t-groups, stacked on partition blocks 32g..32g+27
S = T // G                 # 512 columns per group (= one psum bank)
DCH = D // 128             # 8 contraction chunks
CPC = T + 1                # x cols per chunk incl leading zero column
NSWEEP = 3                 # Jacobi sweeps (sweep 0 is ACT-only)
SCHUNKS = (1, 3, 5, 7)     # chunks whose t-sum runs on Scalar (ACT accum)

import os
K_XQ = os.environ.get("K_XQ", "split")     # x DMA queues: split | sync
K_RED = os.environ.get("K_RED", "fast")    # t-sums: fast (Vec+Scalar) | safe
K_CMM = os.environ.get("K_CMM", "f32r")    # c-matmul weights: f32r | f16


def build_body(nc, xt, w2t, cot4, wmm, eye, yg, tc):
    from contextlib import ExitStack
    ctx = ExitStack()
    sbp = ctx.enter_context(tc.tile_pool(name="sb", bufs=1))
    pp = ctx.enter_context(tc.tile_pool(name="pp", bufs=1, space="PSUM"))

    def st(shape, name, dt=F32):
        return sbp.tile(shape, dt, name=name, tag=name)

    xts = [st([128, CPC], f"xt{c}", F16) for c in range(DCH)]
    w2t_sb = st([128, DCH, 32], "w2t_sb", F16)
    cot_sb = st([128, DCH, 128], "cot_sb",
                F32R if K_CMM == "f32r" else F16)
    wmm_sb = st([128, 2, 128], "wmm_sb", F16)
    eye_sb = st([128, 128], "eye_sb", F16)
    ya = st([128, S + 1], "ya", F16)
    bsb = st([128, S], "bsb", F16)
    scrs = st([128, T], "scrs", F16)
    # per-chunk t-sums in column PAIRS (col 1 zero) so the f32r c-matmul rhs
    # slices stay 8-byte aligned; f32r-typed so the reducers round for the PE
    SDT = F32R if K_CMM == "f32r" else F32
    scolv = st([128, DCH, 2], "scolv", SDT)
    scols = st([128, DCH, 2], "scols", SDT)
    scol16 = st([128, DCH, 2], "scol16", F16)
    spart = st([128, 64], "spart")
    cbias = st([128, 1], "cbias")
    yfin = st([128, S], "yfin")
    dummy = st([1, 1], "dummy")

    psB = pp.tile([128, S], F32, name="psB", tag="psB")
    z1 = pp.tile([128, S], F32, name="z1", tag="z1")
    z2 = pp.tile([128, S], F32, name="z2", tag="z2")
    cps = pp.tile([128, 2], F32, name="cps", tag="cps")

    # Early dummy sigmoid so the ACT table load happens off the critical path.
    nc.vector.memset(dummy[:, :], 0.0)
    nc.scalar.activation(out=dummy[:, :], in_=dummy[:, :], func=AF.Sigmoid)
    nc.vector.memset(ya[:, :].bitcast(U16), 0)
    nc.vector.memset(scolv[:, :, :].bitcast(F32), 0.0)
    nc.vector.memset(scols[:, :, :].bitcast(F32), 0.0)

    # DMAs.  SP queue: w2t (gates the PE) + even chunks + late consts.
    # Activation queue: odd chunks (triggers early, before Scalar's reduces).
    xv = xt.rearrange("p (c t) -> p c t", c=DCH)
    nc.sync.dma_start(w2t_sb[:, :, :], w2t.rearrange("p (c w) -> p c w", c=DCH))
    if K_XQ == "split":
        for c in range(0, DCH, 2):
            nc.sync.dma_start(xts[c][:, :], xv[:, c, :])
        for c in range(1, DCH, 2):
            nc.scalar.dma_start(xts[c][:, :], xv[:, c, :])
    else:
        for c in range(DCH):
            nc.sync.dma_start(xts[c][:, :], xv[:, c, :])
    # late-needed consts ride the idle GpSimd queue so the sync/scalar
    # queues (which gate the U-phase and the t-sums) finish sooner
    nc.gpsimd.dma_start(cot_sb[:, :, :],
                        cot4.rearrange("p (c w) -> p c w", c=DCH))
    nc.gpsimd.dma_start(wmm_sb[:, :, :],
                        wmm.rearrange("p (h q) -> p h q", h=2))
    nc.gpsimd.dma_start(eye_sb[:, :], eye)

    # ---- c-path t-sums ----
    H = T // 2
    with nc.allow_low_precision(reason="t-sum accum in f32r (19-bit) is "
                                "plenty for c; f32r feeds the PE directly"):
        def vec_two_stage(in_ap, dst, chain):
            # fp16 chains of `chain`, then fp32 — the DVE's fused reduce
            # paths mis-accumulate 16-bit inputs on HW
            nc.vector.tensor_reduce(
                out=spart[:, 0:in_ap.shape[-1] // chain],
                in_=in_ap.rearrange("p (a b) -> p a b", b=chain),
                axis=mybir.AxisListType.X, op=ALU.add)
            nc.vector.tensor_reduce(
                out=dst, in_=spart[:, 0:in_ap.shape[-1] // chain],
                axis=mybir.AxisListType.X, op=ALU.add)

        for c in range(DCH):
            if K_RED == "fast" and c in SCHUNKS:
                nc.scalar.activation(out=scrs[:, :], in_=xts[c][:, 1:],
                                     func=AF.Identity,
                                     accum_out=scols[:, c, 0:1])
            else:
                vec_two_stage(xts[c][:, 1:], scolv[:, c, 0:1], 32)

    # ---- U-phase: psB[32g+v, tau] = sum_d Uo[v,d] x[512g+tau-1, d] ----
    for c in range(DCH):
        if c == DCH - 1:
            # c = Co @ s via group-replicated fp32 Co weights (1-col fp32
            # matmuls are free and keep c accurate); before the last
            # chunk's group matmuls so the bias lands off-path.
            use_s = (K_RED == "fast")
            if K_CMM != "f32r":
                nc.vector.tensor_copy(scol16[:, :, :],
                                      scolv[:, :, :].bitcast(F32))
            for cc in range(DCH):
                if K_CMM == "f32r":
                    s = scols if (use_s and cc in SCHUNKS) else scolv
                    rhs = s[:, cc, :]
                else:
                    rhs = scol16[:, cc, :]
                nc.tensor.matmul(
                    cps[:, :], lhsT=cot_sb[:, cc, :], rhs=rhs,
                    start=(cc == 0), stop=(cc == DCH - 1),
                    skip_group_check=True,
                )
            nc.vector.tensor_copy(cbias[:, :], cps[:, 0:1])
        for g in range(G):
            nc.tensor.matmul(
                psB[32 * g:32 * g + 32, :],
                lhsT=w2t_sb[:, c, :],
                rhs=xts[c][:, S * g:S * g + S],
                start=(c == 0), stop=(c == DCH - 1),
                tile_position=(0, 32 * g),
                skip_group_check=True,
            )

    # fp16 copy of B (Scalar ACT-copy: Vector is busy with sums);
    # pre-load the sweep psum banks with it (PE idle then)
    nc.scalar.copy(bsb[:, :], psB[:, :])
    for z in (z1, z2):
        nc.tensor.matmul(z[:, :], lhsT=eye_sb[:, :], rhs=bsb[:, :],
                         start=True, stop=False, skip_group_check=True)

    # ---- Jacobi sweeps ----
    # YA[32g+v, j]: j=0 boundary col (zero; block boundaries flow through the
    # wmm[:,1,:] shift matmul), j>=1 holds y[512g+j-1].
    for k in range(NSWEEP):
        if k == 0:
            nc.scalar.activation(out=ya[:, 1:S + 1], in_=psB[:, :],
                                 func=AF.Sigmoid, bias=cbias[:, 0:1],
                                 scale=1.0)
            continue
        z = z1 if k % 2 == 1 else z2
        nc.tensor.matmul(z[:, :], lhsT=wmm_sb[:, 0, :], rhs=ya[:, 0:S],
                         start=False, stop=False, skip_group_check=True)
        nc.tensor.matmul(z[:, 0:1], lhsT=wmm_sb[:, 1, :], rhs=ya[:, S:S + 1],
                         start=False, stop=True, skip_group_check=True)
        out = ya[:, 1:S + 1] if k < NSWEEP - 1 else yfin[:, :]
        nc.scalar.activation(out=out, in_=z[:, :], func=AF.Sigmoid,
                             bias=cbias[:, 0:1], scale=1.0)

    nc.sync.dma_start(yg, yfin[:, :])
    ctx.close()


_CACHED_NC = {}


def _get_nc():
    if "nc" not in _CACHED_NC:
        nc = bacc.Bacc("TRN2", target_bir_lowering=False, debug=False,
                       num_devices=N_CORES)
        xt = nc.dram_tensor("xt", [128, DCH * CPC], F16, kind="ExternalInput")
        w2t = nc.dram_tensor("w2t", [128, DCH * 32], F16, kind="ExternalInput")
        cot4 = nc.dram_tensor("cot4", [128, DCH * 128],
                              F32R if K_CMM == "f32r" else F16,
                              kind="ExternalInput")
        wmm = nc.dram_tensor("wmm", [128, 2 * 128], F16, kind="ExternalInput")
        eye = nc.dram_tensor("eye", [128, 128], F16, kind="ExternalInput")
        yg = nc.dram_tensor("yg", [128, S], F32, kind="ExternalOutput")
        with tile.TileContext(nc) as tc:
            build_body(nc, xt.ap(), w2t.ap(), cot4.ap(), wmm.ap(), eye.ap(),
                       yg.ap(), tc)
        nc.compile()
        _CACHED_NC["nc"] = nc
    return _CACHED_NC["nc"]


def _diffuse_fp16(xb):
    """Round (T, D) fp32 -> fp16 with 1-D error diffusion along t so that
    column sums are preserved to ~1 ulp (keeps c = Co @ sum_t x[t] accurate
    from the fp16 copy alone)."""
    q = np.empty(xb.shape, np.float16)
    e = np.zeros(xb.shape[1], np.float32)
    for t in range(xb.shape[0]):
        v = xb[t] + e
        qt = v.astype(np.float16)
        e = v - qt.astype(np.float32)
        q[t] = qt
    return q


def make_in_maps(x, Uo, Co, Wo):
    xb = np.asarray(x, np.float32)[0]                              # (T, D)
    xq = _diffuse_fp16(xb)
    xc = np.zeros((128, DCH, CPC), np.float16)
    xc[:, :, 1:] = xq.T.reshape(DCH, 128, T).transpose(1, 0, 2)
    xc = np.ascontiguousarray(xc.reshape(128, DCH * CPC))

    w2 = np.zeros((32, D), np.float32)
    w2[0:V] = np.asarray(Uo, np.float32)
    w2t = np.ascontiguousarray(
        w2.T.reshape(DCH, 128, 32).transpose(1, 0, 2)
        .reshape(128, DCH * 32)).astype(np.float16)

    c4 = np.zeros((128, D), np.float32)                            # (4*32, D)
    for g in range(G):
        c4[32 * g:32 * g + V] = np.asarray(Co, np.float32)
    cot4 = np.ascontiguousarray(
        c4.T.reshape(DCH, 128, 128).transpose(1, 0, 2)
        .reshape(128, DCH * 128))
    if K_CMM != "f32r":
        cot4 = cot4.astype(np.float16)

    wot = np.asarray(Wo, np.float32).T                             # (v, w)
    wmm = np.zeros((128, 2, 128), np.float16)
    for g in range(G):
        wmm[32 * g:32 * g + V, 0, 32 * g:32 * g + V] = wot
        if g > 0:
            wmm[32 * (g - 1):32 * (g - 1) + V, 1, 32 * g:32 * g + V] = wot
    wmm = np.ascontiguousarray(wmm.reshape(128, 2 * 128))

    eye = np.eye(128, dtype=np.float16)

    m = {"xt": xc, "w2t": w2t, "cot4": cot4, "wmm": wmm, "eye": eye}
    return [m] * N_CORES


def unshard_output(yg):
    y = np.empty((T, V), np.float32)
    for g in range(G):
        y[g * S:(g + 1) * S, :] = yg[32 * g:32 * g + V, :].T
    return y[None]


def run(inputs, trace=False, **kw):
    nc = _get_nc()
    in_maps = make_in_maps(inputs["x"], inputs["Uo"], inputs["Co"],
                           inputs["Wo"])
    res = bass_utils.run_bass_kernel_spmd(
        nc, in_maps, core_ids=list(range(N_CORES)), trace=trace, **kw)
    return unshard_output(res.results[0]["yg"]), res


def kernel(**inputs):
    out, _ = run(inputs)
    return out


# revision 51
# speedup vs baseline: 1.0890x; 1.0890x over previous
"""Trainium2 Bass kernel for nn_CascadedAttention_76836964925817.

Math: the reference module's attention machinery is dead code — softmax over a
size-1 axis is identically 1, so `context = x[0].sum(axis=0)` is a constant
and the layer reduces to the 28-dim nonlinear recurrence

    y[t] = sigmoid(Wo @ y[t-1] + Uo @ x[t-1] + c),   c = Co @ sum_t x[t],
    y[-1] = 0, x[-1] := 0,

solved by Jacobi fixed-point sweeps (the map is a strong contraction:
|sigmoid'| <= 1/4, ||Wo|| ~ 0.53, plus heavy sigmoid saturation from the
large |c|; 3 sweeps reach the fp16 data floor of ~1e-3 rel).

Strategy: **no collectives**.  Every core redundantly computes the whole
problem from an fp16 copy of x (4.2 MB); the host reads core 0's output.
x is rounded on the host with 1-D error diffusion along t (per feature), so
the device-side t-sums reproduce the fp32 sums to ~1 ulp — this keeps the
constant c accurate without shipping a second hi/lo copy of x.

  * x is d-major, one (128, 1+2048) fp16 tile per 128-wide d-chunk (one
    leading zero column so B's shift-by-one is a column offset); chunk DMAs
    split across the two HWDGE queues (SP even chunks, Activation odd) with
    per-chunk tiles so consumers start as soon as their chunk lands.
  * U-phase: per chunk, 4 matmuls (lhsT = padded Uo chunk (128, 32)) write
    the four 512-col t-groups into ONE stacked psum bank at 32-partition
    strides (tile_position=(0, 32g)) — the bank then IS the shifted B.
  * c-path: per-chunk t-sums — 6 chunks on Vector (fused pair-add + reduce,
    fp32 elementwise output), 2 on Scalar (Identity activation with fp32
    accum_out); then 8 tiny accumulating matmuls against group-replicated
    Co weights (128-wide lhsT) put the full 128-partition bias in one psum
    column; GpSimd copies it to SBUF.
  * Sweeps: sweep 0 is one ACT (sigmoid(B + c) straight from the U psum).
    The two z psum banks are pre-loaded with B (eye matmuls, off-path);
    sweeps 1-2 accumulate blockdiag(Wo^T) @ YA (plus a 1-col group-boundary
    matmul) on top and ACT with bias=c.

The kernel is self-contained: shapes/sharding are hardcoded.
"""

import numpy as np

import concourse.bass as bass
import concourse.mybir as mybir
import concourse.tile as tile
from concourse import bacc
from concourse import bass_utils

F32 = mybir.dt.float32
F32R = mybir.dt.float32r
F16 = mybir.dt.float16
U16 = mybir.dt.uint16
AF = mybir.ActivationFunctionType
ALU = mybir.AluOpType

T, D, V = 2048, 1024, 28
N_CORES = 8
G = 4                      # t-groups, stacked on partition blocks 32g..32g+27
S = T // G                 # 512 columns per group (= one psum bank)
DCH = D // 128             # 8 contraction chunks
CPC = T + 1                # x cols per chunk incl leading zero column
NSWEEP = 3                 # Jacobi sweeps (sweep 0 is ACT-only)
SCHUNKS = (1, 3, 5)        # chunks whose t-sum runs on Scalar (ACT accum)

import os
K_XQ = os.environ.get("K_XQ", "split")     # x DMA queues: split | sync
K_RED = os.environ.get("K_RED", "fast")    # t-sums: fast (Vec+Scalar) | safe
K_CMM = os.environ.get("K_CMM", "f32r")    # c-matmul weights: f32r | f16


def build_body(nc, xt, w2t, cot4, wmm, eye, yg, tc):
    from contextlib import ExitStack
    ctx = ExitStack()
    sbp = ctx.enter_context(tc.tile_pool(name="sb", bufs=1))
    pp = ctx.enter_context(tc.tile_pool(name="pp", bufs=1, space="PSUM"))

    def st(shape, name, dt=F32):
        return sbp.tile(shape, dt, name=name, tag=name)

    xts = [st([128, CPC], f"xt{c}", F16) for c in range(DCH)]
    w2t_sb = st([128, DCH, 32], "w2t_sb", F16)
    cot_sb = st([128, DCH, 128], "cot_sb",
                F32R if K_CMM == "f32r" else F16)
    wmm_sb = st([128, 2, 128], "wmm_sb", F16)
    eye_sb = st([128, 128], "eye_sb", F16)
    ya = st([128, S + 1], "ya", F16)
    bsb = st([128, S], "bsb", F16)
    scrs = st([128, T], "scrs", F16)
    # per-chunk t-sums in column PAIRS (col 1 zero) so the f32r c-matmul rhs
    # slices stay 8-byte aligned; f32r-typed so the reducers round for the PE
    SDT = F32R if K_CMM == "f32r" else F32
    scolv = st([128, DCH, 2], "scolv", SDT)
    scols = st([128, DCH, 2], "scols", SDT)
    scol16 = st([128, DCH, 2], "scol16", F16)
    spart = st([128, 64], "spart")
    cbias = st([128, 1], "cbias")
    yfin = st([128, S], "yfin")
    dummy = st([1, 1], "dummy")

    psB = pp.tile([128, S], F32, name="psB", tag="psB")
    z1 = pp.tile([128, S], F32, name="z1", tag="z1")
    z2 = pp.tile([128, S], F32, name="z2", tag="z2")
    cps = pp.tile([128, 2], F32, name="cps", tag="cps")

    # Early dummy sigmoid so the ACT table load happens off the critical path.
    nc.vector.memset(dummy[:, :], 0.0)
    nc.scalar.activation(out=dummy[:, :], in_=dummy[:, :], func=AF.Sigmoid)
    nc.vector.memset(ya[:, :].bitcast(U16), 0)
    nc.vector.memset(scolv[:, :, :].bitcast(F32), 0.0)
    nc.vector.memset(scols[:, :, :].bitcast(F32), 0.0)

    # DMAs.  SP queue: w2t (gates the PE) + even chunks + late consts.
    # Activation queue: odd chunks (triggers early, before Scalar's reduces).
    xv = xt.rearrange("p (c t) -> p c t", c=DCH)
    nc.sync.dma_start(w2t_sb[:, :, :], w2t.rearrange("p (c w) -> p c w", c=DCH))
    if K_XQ == "split":
        for c in range(0, DCH, 2):
            nc.sync.dma_start(xts[c][:, :], xv[:, c, :])
        for c in range(1, DCH, 2):
            nc.scalar.dma_start(xts[c][:, :], xv[:, c, :])
    else:
        for c in range(DCH):
            nc.sync.dma_start(xts[c][:, :], xv[:, c, :])
    nc.sync.dma_start(cot_sb[:, :, :],
                      cot4.rearrange("p (c w) -> p c w", c=DCH))
    nc.sync.dma_start(wmm_sb[:, :, :], wmm.rearrange("p (h q) -> p h q", h=2))
    nc.sync.dma_start(eye_sb[:, :], eye)

    # ---- c-path t-sums ----
    H = T // 2
    with nc.allow_low_precision(reason="t-sum accum in f32r (19-bit) is "
                                "plenty for c; f32r feeds the PE directly"):
        def vec_two_stage(in_ap, dst, chain):
            # fp16 chains of `chain`, then fp32 — the DVE's fused reduce
            # paths mis-accumulate 16-bit inputs on HW
            nc.vector.tensor_reduce(
                out=spart[:, 0:in_ap.shape[-1] // chain],
                in_=in_ap.rearrange("p (a b) -> p a b", b=chain),
                axis=mybir.AxisListType.X, op=ALU.add)
            nc.vector.tensor_reduce(
                out=dst, in_=spart[:, 0:in_ap.shape[-1] // chain],
                axis=mybir.AxisListType.X, op=ALU.add)

        for c in range(DCH):
            if K_RED == "fast" and c in SCHUNKS:
                nc.scalar.activation(out=scrs[:, :], in_=xts[c][:, 1:],
                                     func=AF.Identity,
                                     accum_out=scols[:, c, 0:1])
            else:
                vec_two_stage(xts[c][:, 1:], scolv[:, c, 0:1], 32)

    # ---- U-phase: psB[32g+v, tau] = sum_d Uo[v,d] x[512g+tau-1, d] ----
    for c in range(DCH):
        if c == DCH - 1:
            # c = Co @ s via group-replicated fp32 Co weights (1-col fp32
            # matmuls are free and keep c accurate); before the last
            # chunk's group matmuls so the bias lands off-path.
            use_s = (K_RED == "fast")
            if K_CMM != "f32r":
                nc.vector.tensor_copy(scol16[:, :, :],
                                      scolv[:, :, :].bitcast(F32))
            for cc in range(DCH):
                if K_CMM == "f32r":
                    s = scols if (use_s and cc in SCHUNKS) else scolv
                    rhs = s[:, cc, :]
                else:
                    rhs = scol16[:, cc, :]
                nc.tensor.matmul(
                    cps[:, :], lhsT=cot_sb[:, cc, :], rhs=rhs,
                    start=(cc == 0), stop=(cc == DCH - 1),
                    skip_group_check=True,
                )
            nc.vector.tensor_copy(cbias[:, :], cps[:, 0:1])
        for g in range(G):
            nc.tensor.matmul(
                psB[32 * g:32 * g + 32, :],
                lhsT=w2t_sb[:, c, :],
                rhs=xts[c][:, S * g:S * g + S],
                start=(c == 0), stop=(c == DCH - 1),
                tile_position=(0, 32 * g),
                skip_group_check=True,
            )

    # fp16 copy of B (Scalar ACT-copy: Vector is busy with sums);
    # pre-load the sweep psum banks with it (PE idle then)
    nc.scalar.copy(bsb[:, :], psB[:, :])
    for z in (z1, z2):
        nc.tensor.matmul(z[:, :], lhsT=eye_sb[:, :], rhs=bsb[:, :],
                         start=True, stop=False, skip_group_check=True)

    # ---- Jacobi sweeps ----
    # YA[32g+v, j]: j=0 boundary col (zero; block boundaries flow through the
    # wmm[:,1,:] shift matmul), j>=1 holds y[512g+j-1].
    for k in range(NSWEEP):
        if k == 0:
            nc.scalar.activation(out=ya[:, 1:S + 1], in_=psB[:, :],
                                 func=AF.Sigmoid, bias=cbias[:, 0:1],
                                 scale=1.0)
            continue
        z = z1 if k % 2 == 1 else z2
        nc.tensor.matmul(z[:, :], lhsT=wmm_sb[:, 0, :], rhs=ya[:, 0:S],
                         start=False, stop=False, skip_group_check=True)
        nc.tensor.matmul(z[:, 0:1], lhsT=wmm_sb[:, 1, :], rhs=ya[:, S:S + 1],
                         start=False, stop=True, skip_group_check=True)
        out = ya[:, 1:S + 1] if k < NSWEEP - 1 else yfin[:, :]
        nc.scalar.activation(out=out, in_=z[:, :], func=AF.Sigmoid,
                             bias=cbias[:, 0:1], scale=1.0)

    nc.sync.dma_start(yg, yfin[:, :])
    ctx.close()


_CACHED_NC = {}


def _get_nc():
    if "nc" not in _CACHED_NC:
        nc = bacc.Bacc("TRN2", target_bir_lowering=False, debug=False,
                       num_devices=N_CORES)
        xt = nc.dram_tensor("xt", [128, DCH * CPC], F16, kind="ExternalInput")
        w2t = nc.dram_tensor("w2t", [128, DCH * 32], F16, kind="ExternalInput")
        cot4 = nc.dram_tensor("cot4", [128, DCH * 128],
                              F32R if K_CMM == "f32r" else F16,
                              kind="ExternalInput")
        wmm = nc.dram_tensor("wmm", [128, 2 * 128], F16, kind="ExternalInput")
        eye = nc.dram_tensor("eye", [128, 128], F16, kind="ExternalInput")
        yg = nc.dram_tensor("yg", [128, S], F32, kind="ExternalOutput")
        with tile.TileContext(nc) as tc:
            build_body(nc, xt.ap(), w2t.ap(), cot4.ap(), wmm.ap(), eye.ap(),
                       yg.ap(), tc)
        nc.compile()
        _CACHED_NC["nc"] = nc
    return _CACHED_NC["nc"]


def _diffuse_fp16(xb):
    """Round (T, D) fp32 -> fp16 with 1-D error diffusion along t so that
    column sums are preserved to ~1 ulp (keeps c = Co @ sum_t x[t] accurate
    from the fp16 copy alone)."""
    q = np.empty(xb.shape, np.float16)
    e = np.zeros(xb.shape[1], np.float32)
    for t in range(xb.shape[0]):
        v = xb[t] + e
        qt = v.astype(np.float16)
        e = v - qt.astype(np.float32)
        q[t] = qt
    return q


def make_in_maps(x, Uo, Co, Wo):
    xb = np.asarray(x, np.float32)[0]                              # (T, D)
    xq = _diffuse_fp16(xb)
    xc = np.zeros((128, DCH, CPC), np.float16)
    xc[:, :, 1:] = xq.T.reshape(DCH, 128, T).transpose(1, 0, 2)
    xc = np.ascontiguousarray(xc.reshape(128, DCH * CPC))

    w2 = np.zeros((32, D), np.float32)
    w2[0:V] = np.asarray(Uo, np.float32)
    w2t = np.ascontiguousarray(
        w2.T.reshape(DCH, 128, 32).transpose(1, 0, 2)
        .reshape(128, DCH * 32)).astype(np.float16)

    c4 = np.zeros((128, D), np.float32)                            # (4*32, D)
    for g in range(G):
        c4[32 * g:32 * g + V] = np.asarray(Co, np.float32)
    cot4 = np.ascontiguousarray(
        c4.T.reshape(DCH, 128, 128).transpose(1, 0, 2)
        .reshape(128, DCH * 128))
    if K_CMM != "f32r":
        cot4 = cot4.astype(np.float16)

    wot = np.asarray(Wo, np.float32).T                             # (v, w)
    wmm = np.zeros((128, 2, 128), np.float16)
    for g in range(G):
        wmm[32 * g:32 * g + V, 0, 32 * g:32 * g + V] = wot
        if g > 0:
            wmm[32 * (g - 1):32 * (g - 1) + V, 1, 32 * g:32 * g + V] = wot
    wmm = np.ascontiguousarray(wmm.reshape(128, 2 * 128))

    eye = np.eye(128, dtype=np.float16)

    m = {"xt": xc, "w2t": w2t, "cot4": cot4, "wmm": wmm, "eye": eye}
    return [m] * N_CORES


def unshard_output(yg):
    y = np.empty((T, V), np.float32)
    for g in range(G):
        y[g * S:(g + 1) * S, :] = yg[32 * g:32 * g + V, :].T
    return y[None]


def run(inputs, trace=False, **kw):
    nc = _get_nc()
    in_maps = make_in_maps(inputs["x"], inputs["Uo"], inputs["Co"],
                           inputs["Wo"])
    res = bass_utils.run_bass_kernel_spmd(
        nc, in_maps, core_ids=list(range(N_CORES)), trace=trace, **kw)
    return unshard_output(res.results[0]["yg"]), res


def kernel(**inputs):
    out, _ = run(inputs)
    return out
